# revision 1
# baseline (speedup 1.0000x reference)
"""Trainium2 Bass kernel for DynamicSpectralTilt IIR.

Math (from the reference nn.Module):
    u     = log2(f0 / 1200 + 1)                      (nyquist=12000, *10)
    z     = w2 @ leaky_relu(w1 * u + b1, 0.2) + b2   (pointwise MLP, hidden=64)
    alpha = 0.98 * sigmoid(z) * voiced_mask
    y[t]  = alpha[t] * y[t-1] + (1 - alpha[t]) * x[t]   (first-order IIR)

Device strategy (8 cores, batch-parallel, 2 batch elements per core):
  * z(f0) is a fixed scalar function of f0 determined by the (tiny, runtime)
    weights.  It is fit on the host with a low-degree polynomial in
    s = (f0 - mid)/half over the observed f0 range; the alpha error of the
    fit is ~2e-5 (the sigmoid squashes z-error by ~50x).  On device it is a
    short Horner chain (DVE) + one ACT sigmoid.
  * The IIR uses the hardware tensor_tensor_scan (DVE) along the free dim:
    each T=524288 sequence is laid out as [128 partitions x 4096]; each
    partition scans its own chunk.  alpha <= 0.98*sigmoid(z_max) is small,
    so the product of alphas over a 4096-chunk underflows to exactly 0 in
    fp32 and the true carry into chunk p is exactly the last scan value of
    chunk p-1.  A cumprod over the first `win` columns applies the carry:
        y[p, i] = Z[p, i] + cumprod_alpha[p, i] * carry[p],  i < win.
    Partition 0 needs no correction: initial_state is fed to the scan
    directly via its per-partition `initial` operand.

Compiler constraint: this toolchain rejects instructions with more than ~2
sync commands (1 wait + 1 self update), so the program is arranged so every
instruction's operand producers collapse onto a single semaphore: fresh tile
slots everywhere (no WAR/WAW waits), one same-engine "pre-touch" op per
DMA-loaded tile, and single-engine producer sets for every consumer.
"""

import numpy as np

_B, _T = 16, 524288
_NCORES = 8
_BPC = _B // _NCORES          # batch elements per core
_P = 128                      # SBUF partitions (chunks per sequence)
_L = _T // _P                 # 4096 columns per partition
_NCHUNK = 8                   # free-dim chunks per batch element
_NYQ = 12000.0                # SAMPLE_RATE / 2
_K = 10.0 / _NYQ


def _exact_z(f0, w1, b1, w2, b2):
    """Reference MLP z(f0) in float64 on the host (f0: 1-D array)."""
    u = np.log2(f0 * _K + 1.0)
    h = w1.reshape(-1, 1).astype(np.float64) * u[None, :] + b1.reshape(-1, 1).astype(
        np.float64
    )
    h = np.where(h >= 0.0, h, 0.2 * h)
    return w2.reshape(-1).astype(np.float64) @ h + float(np.asarray(b2).reshape(-1)[0])


def _fit_poly(w1, b1, w2, b2, fmin, fmax, deg, in_scale=1.0):
    """Least-squares polynomial fit of z in the device variable g=f0*in_scale.

    Returns (coef ascending p[0..deg], gmid, ghalf, max_alpha, alpha_fit_err).
    """
    fmid = 0.5 * (fmin + fmax) * in_scale
    fhalf = max(0.5 * (fmax - fmin) * in_scale, 1e-3)
    grid = np.linspace(fmin * in_scale, fmax * in_scale, 200001)
    zg = _exact_z(grid / in_scale, w1, b1, w2, b2)
    sg = (grid - fmid) / fhalf
    cheb = np.polynomial.chebyshev.Chebyshev.fit(sg, zg, deg, domain=[-1, 1])
    p = np.polynomial.chebyshev.cheb2poly(cheb.coef)
    pv = np.polynomial.polynomial.polyval(sg, p)
    ag = 0.98 / (1.0 + np.exp(-zg))
    af = 0.98 / (1.0 + np.exp(-pv))
    # compose with s(f0) so the device evaluates Horner directly in f0
    # (conditioning verified: fp32 Horner error is unchanged)
    comp = np.polynomial.polynomial.Polynomial([-fmid / fhalf, 1.0 / fhalf])
    pf = np.polynomial.polynomial.Polynomial(p)(comp).coef
    return pf, fmid, fhalf, float(ag.max()), float(np.abs(af - ag).max())


def _numpy_fallback(x, f0, vm, y0, w1, b1, w2, b2):
    """Exact (sequential, fp32) host computation.  Safety net only."""
    f32 = np.float32
    z = _exact_z(f0.reshape(-1).astype(np.float64), w1, b1, w2, b2).reshape(f0.shape)
    alpha = (0.98 / (1.0 + np.exp(-z)) * vm.astype(np.float64)).astype(f32)
    beta = ((f32(1.0) - alpha) * x.astype(f32)).astype(f32)
    B = x.shape[0]
    T = x.shape[-1]
    st = y0.reshape(B).astype(f32).copy()
    y = np.empty_like(x, dtype=f32)
    a2 = alpha.reshape(B, T)
    b2_ = beta.reshape(B, T)
    yv = y.reshape(B, T)
    for t in range(T):
        st = (a2[:, t] * st + b2_[:, t]).astype(f32)
        yv[:, t] = st
    return y.reshape(x.shape)


def _build_bass(poly, fmid, fhalf, win, P, L, nchunk, bpc, use_bacc=True,
                pool_ks=None, a2_pool=False, alpha_pool=True, nb_pool=False,
                ring_split=False, reps=1, dma_whole=False, bf16_in=False,
                fs_pack=False):
    """Build the per-core Bass program (same program for all cores).

    use_bacc: build on the Bacc layer, whose finalize() legalizes sync waits
    (at most 1 wait per instruction) via EventSemaphore splitting.  CoreSim
    tests pass False and interpret the raw Bass stream instead.
    """
    import concourse.bass as bass
    import concourse.mybir as mybir
    from concourse.tile import TileContext

    f32 = mybir.dt.float32
    Alu = mybir.AluOpType
    Act = mybir.ActivationFunctionType
    CW = L // nchunk

    D = len(poly) - 1
    p = [float(v) for v in poly]

    if use_bacc:
        from concourse.bacc import Bacc

        nc = Bacc()
    else:
        nc = bass.Bass()
    if fs_pack:
        # x stays fp32; f0 and voiced_mask are packed into ONE narrow plane:
        # fs = vm ? f0*s : -f0*s (f0 > 0 guaranteed by the caller; the scale
        # is folded into the poly fit).  The device recovers |fs| via ACT Abs
        # and the mask via is_gt(fs, 0).  fs_pack == "fp8" uses e4m3.
        fs_dt = mybir.dt.float8e4 if fs_pack == "fp8" else mybir.dt.bfloat16
        xin_d = nc.declare_dram_parameter("xin", [bpc, P, L], f32, False)
        fs_d = nc.declare_dram_parameter("fs", [bpc, P, L], fs_dt, False)
    elif bf16_in:
        # x stays fp32 (output precision); f0 and voiced_mask are host-downcast
        # to bf16 (alpha error ~3e-6; mask 0/1 exact) to cut DMA traffic 27%.
        bf16 = mybir.dt.bfloat16
        xin_d = nc.declare_dram_parameter("xin", [bpc, P, L], f32, False)
        fv_d = nc.declare_dram_parameter("fv", [bpc, P, 2, L], bf16, False)
    else:
        # f0/x/vm host-interleaved as [bpc, P, 3, L] so each chunk needs ONE
        # load DMA (HWDGE descriptor generation dominates small chunks)
        fxm_d = nc.declare_dram_parameter("fxm", [bpc, P, 3, L], f32, False)
    # y0 is host-padded to [P, 1]: row 0 = initial_state, rows 1.. = 0.
    y0_d = nc.declare_dram_parameter("y0", [bpc, P, 1], f32, False)
    y_d = nc.declare_dram_parameter("y", [bpc, P, L], f32, True)

    with TileContext(nc) as tc:
        with (
            tc.tile_pool(name="big", bufs=2) as pool,
            tc.tile_pool(name="small", bufs=2) as spool,
        ):
            zeros_w = spool.tile([P, win], f32, tag="zw", bufs=1)
            nc.vector.memset(zeros_w, 0.0)
            bias_t = spool.tile([P, 1], f32, tag="bias", bufs=1)
            nc.vector.memset(bias_t, p[0])

            import contextlib

            nbuf = bpc * nchunk
            rep_ctx = (
                tc.For_i(0, reps, 1) if reps > 1 else contextlib.nullcontext()
            )
            with rep_ctx:
              for e in range(bpc):
                # per-partition scan initial state: [y0, 0, 0, ...]
                INIT = spool.tile([P, 1], f32, tag="init", bufs=bpc)
                nc.sync.dma_start(out=INIT, in_=y0_d[e])
                INITV = spool.tile([P, 1], f32, tag="initv", bufs=bpc)
                nc.vector.tensor_scalar_mul(INITV, INIT, 1.0)  # absorb DMA wait

                z_first = None
                z_prev = None
                a64 = None
                if fs_pack:
                    # fs on the ACT ring, x on the SP ring: the two HWDGE
                    # queues stream concurrently (x 4MB vs fs 2MB + stores)
                    TFS = pool.tile([P, L], fs_dt, tag="sep_fs", bufs=2)
                    nc.scalar.dma_start(out=TFS, in_=fs_d[e])
                    TX = pool.tile([P, L], f32, tag="sep_x", bufs=2)
                    nc.sync.dma_start(out=TX, in_=xin_d[e])
                    ZW = pool.tile([P, L], f32, tag="zw_full", bufs=2)
                elif bf16_in:
                    TFV = pool.tile([P, 2 * L], mybir.dt.bfloat16, tag="sep_fv", bufs=2)
                    nc.sync.dma_start(out=TFV, in_=fv_d[e])
                    TX = pool.tile([P, L], f32, tag="sep_x", bufs=2)
                    nc.sync.dma_start(out=TX, in_=xin_d[e])
                    TF = TFV[:, 0:L]
                    TM = TFV[:, L : 2 * L]
                    ZW = pool.tile([P, L], f32, tag="zw_full", bufs=2)
                elif dma_whole == "split3":
                    # three big per-tensor loads (16KB contiguous per
                    # partition): poly work starts after the f0 load lands
                    # instead of after the whole 6MB block.
                    TF = pool.tile([P, L], f32, tag="sep_f", bufs=2)
                    nc.sync.dma_start(out=TF, in_=fxm_d[e][:, 0, :])
                    TX = pool.tile([P, L], f32, tag="sep_x", bufs=2)
                    nc.sync.dma_start(out=TX, in_=fxm_d[e][:, 1, :])
                    TM = pool.tile([P, L], f32, tag="sep_m", bufs=2)
                    nc.sync.dma_start(out=TM, in_=fxm_d[e][:, 2, :])
                    ZW = pool.tile([P, L], f32, tag="zw_full", bufs=2)
                elif dma_whole:
                    # one 48KB-per-partition contiguous load per element; the
                    # compute below slices chunk views out of it.  Small
                    # per-partition DMA segments measure far below line rate.
                    TW = pool.tile([P, 3 * L], f32, tag="fxm", bufs=2)
                    nc.sync.dma_start(out=TW, in_=fxm_d[e])
                    ZW = pool.tile([P, L], f32, tag="zw_full", bufs=2)
                for ci in range(nchunk):
                    lo, hi = ci * CW, (ci + 1) * CW
                    if fs_pack:
                        X = TX[:, lo:hi]
                        FS = TFS[:, lo:hi]
                        # |fs| (fp32) for the Horner chain  [ACT]
                        F = pool.tile([P, CW], f32, tag="absf", bufs=nbuf)
                        nc.scalar.activation(out=F, in_=FS, func=Act.Abs)
                        M = None  # built later from the sign, after F is dead
                    elif bf16_in or dma_whole == "split3":
                        F = TF[:, lo:hi]
                        X = TX[:, lo:hi]
                        M = TM[:, lo:hi]
                    elif dma_whole:
                        F = TW[:, lo:hi]
                        X = TW[:, L + lo : L + hi]
                        M = TW[:, 2 * L + lo : 2 * L + hi]
                    else:
                        ld = nc.scalar if (ring_split and ci % 2) else nc.sync
                        T = pool.tile([P, 3 * CW], f32, tag="fxm", bufs=nbuf)
                        ld.dma_start(out=T, in_=fxm_d[e][:, :, lo:hi])
                        F = T[:, 0:CW]
                        X = T[:, CW : 2 * CW]
                        M = T[:, 2 * CW : 3 * CW]

                    # Horner chain for z - p[0], directly in f0 (coefficients
                    # pre-composed with s=(f0-mid)/half on the host):
                    #   A = p[D]*f + p[D-1]; A *= f; A = (A + p[D-k+1]) * f ...
                    # GPSIMD(Pool) only supports tensor_tensor add/mult, so
                    # Pool offload is limited to the pure-multiply steps.
                    A = pool.tile([P, CW], f32, tag="acc", bufs=nbuf)
                    nc.vector.tensor_scalar(A, F, p[D], p[D - 1], Alu.mult, Alu.add)
                    eng = nc.gpsimd if a2_pool else nc.vector
                    eng.tensor_tensor(out=A, in0=A, in1=F, op=Alu.mult)
                    for k in range(3, D + 1):
                        nc.vector.scalar_tensor_tensor(
                            out=A, in0=A, scalar=p[D - k + 1], in1=F,
                            op0=Alu.add, op1=Alu.mult,
                        )
                    if fs_pack:
                        # mask*0.98 = (fs > 0) * 0.98 in one DVE tensor_scalar,
                        # written over the |fs| tile (dead after the Horner)
                        MP = F
                        nc.vector.tensor_scalar(
                            MP, FS, 0.0, 0.98, Alu.is_gt, Alu.mult
                        )
                    else:
                        # 0.98 * mask on ACT (slack engine)
                        MP = M
                        nc.scalar.mul(MP, M, 0.98)
                    # sigma = sigmoid(z) ; p[0] folded into the ACT bias  [ACT]
                    nc.scalar.activation(
                        out=A, in_=A, func=Act.Sigmoid, bias=bias_t[:, 0:1]
                    )
                    # alpha = sigma * (0.98 * mask)
                    eng = nc.gpsimd if alpha_pool else nc.vector
                    eng.tensor_tensor(out=A, in0=A, in1=MP, op=Alu.mult)
                    # nb = (alpha - 1) * x ( = -(1-alpha)x ), in place over x
                    NB = X
                    if nb_pool:
                        T1 = MP  # mask tile is dead after alpha
                        nc.gpsimd.tensor_tensor(out=T1, in0=A, in1=X, op=Alu.mult)
                        nc.gpsimd.tensor_tensor(out=NB, in0=T1, in1=X, op=Alu.subtract)
                    else:
                        nc.vector.scalar_tensor_tensor(
                            out=NB, in0=A, scalar=1.0, in1=X,
                            op0=Alu.subtract, op1=Alu.mult,
                        )
                    # Z scan: state = alpha*state - nb  [DVE]
                    if dma_whole:
                        Z = ZW[:, lo:hi]
                    else:
                        Z = pool.tile([P, CW], f32, tag="z", bufs=nbuf)
                    init = INITV if ci == 0 else z_prev[:, CW - 1 : CW]
                    nc.vector.tensor_tensor_scan(
                        out=Z, data0=A, data1=NB, initial=init,
                        op0=Alu.mult, op1=Alu.subtract,
                    )
                    if ci == 0:
                        # cumprod of alpha over the correction window
                        a64 = spool.tile([P, win], f32, tag="a64", bufs=bpc)
                        nc.vector.tensor_tensor_scan(
                            out=a64, data0=A[:, :win], data1=zeros_w, initial=1.0,
                            op0=Alu.mult, op1=Alu.add,
                        )
                        z_first = Z
                    elif not dma_whole:
                        nc.scalar.dma_start(out=y_d[e][:, lo:hi], in_=Z)
                    z_prev = Z

                # carry[p] = Z_end[p-1] for p>=1 (exact: the full-chunk alpha
                # product underflows to 0 in fp32).  Partition 0 got its
                # initial state through the scan directly.
                C = spool.tile([P, 1], f32, tag="carry", bufs=bpc)
                nc.vector.memset(C, 0.0)  # partition 0 carry stays 0
                nc.sync.dma_start(
                    out=C[1:P, :], in_=z_prev[0 : P - 1, CW - 1 : CW]
                )
                # y[:, :win] += cumprod * carry   (carry[0] == 0; DVE-producer
                # deps are same-engine and free, so this waits only on the
                # carry DMA)
                nc.vector.scalar_tensor_tensor(
                    out=z_first[:, :win], in0=a64, scalar=C,
                    in1=z_first[:, :win], op0=Alu.mult, op1=Alu.add,
                )
                if dma_whole:
                    nc.scalar.dma_start(out=y_d[e], in_=ZW)
                else:
                    nc.scalar.dma_start(out=y_d[e][:, 0:CW], in_=z_first)
    return nc


def kernel(x, f0_upsampled, voiced_mask, initial_state, w1, b1, w2, b2):
    x = np.ascontiguousarray(np.asarray(x, dtype=np.float32))
    f0 = np.ascontiguousarray(np.asarray(f0_upsampled, dtype=np.float32))
    vm = np.ascontiguousarray(np.asarray(voiced_mask, dtype=np.float32))
    y0 = np.ascontiguousarray(np.asarray(initial_state, dtype=np.float32))
    w1 = np.asarray(w1, dtype=np.float32)
    b1 = np.asarray(b1, dtype=np.float32)
    w2 = np.asarray(w2, dtype=np.float32)
    b2 = np.asarray(b2, dtype=np.float32)

    assert x.shape == (_B, 1, _T), x.shape

    fmin, fmax = float(f0.min()), float(f0.max())
    deg = 3
    poly, fmid, fhalf, amax, aerr = _fit_poly(w1, b1, w2, b2, fmin, fmax, deg)
    while aerr > 5e-4 and deg < 11:
        deg += 2
        poly, fmid, fhalf, amax, aerr = _fit_poly(w1, b1, w2, b2, fmin, fmax, deg)

    if amax > 0.9 or aerr > 5e-4:
        # The fast path's carry truncation / poly fit is not safe for these
        # weights; fall back to an exact host computation.
        return _numpy_fallback(x, f0, vm, y0, w1, b1, w2, b2)

    # correction window: alpha^win < 2^-150 (fp32 exact zero), padded up.
    win = int(np.ceil(150.0 * np.log(2.0) / -np.log(max(amax, 1e-6))))
    win = int(min(max(64, ((win + 31) // 32) * 32), 512))

    from concourse.bass_utils import run_bass_kernel_spmd

    import ml_dtypes

    # Pack voiced_mask into f0's sign bit when f0 is strictly positive
    # (one bf16 plane instead of two); otherwise fall back to two planes.
    use_fs = fmin > 0.0
    nc = _build_bass(
        poly, fmid, fhalf, win, _P, _L, _NCHUNK, _BPC,
        a2_pool=True, alpha_pool=False, nb_pool=False,
        dma_whole=True, bf16_in=not use_fs, fs_pack=use_fs,
    )
    nc.finalize()

    in_maps = []
    for c in range(_NCORES):
        sl = slice(c * _BPC, (c + 1) * _BPC)
        y0pad = np.zeros((_BPC, _P, 1), dtype=np.float32)
        y0pad[:, 0, 0] = y0[sl, 0, 0]
        m = {
            "xin": np.ascontiguousarray(x[sl, 0].reshape(_BPC, _P, _L)),
            "y0": y0pad,
        }
        f0s = f0[sl, 0].reshape(_BPC, _P, _L)
        vms = vm[sl, 0].reshape(_BPC, _P, _L)
        if use_fs:
            m["fs"] = np.where(vms > 0.5, f0s, -f0s).astype(ml_dtypes.bfloat16)
        else:
            fv = np.empty((_BPC, _P, 2, _L), dtype=ml_dtypes.bfloat16)
            fv[:, :, 0, :] = f0s.astype(ml_dtypes.bfloat16)
            fv[:, :, 1, :] = vms.astype(ml_dtypes.bfloat16)
            m["fv"] = fv
        in_maps.append(m)

    res = run_bass_kernel_spmd(nc, in_maps, list(range(_NCORES)))
    out = np.empty((_B, 1, _T), dtype=np.float32)
    for c in range(_NCORES):
        out[c * _BPC : (c + 1) * _BPC, 0] = res.results[c]["y"].reshape(_BPC, _T)
    return out



# revision 2
# speedup vs baseline: 2.2678x; 2.2678x over previous
"""Trainium2 Bass kernel v2 for DynamicSpectralTilt IIR.

Math (from the reference nn.Module):
    u     = log2(f0 / 1200 + 1)                      (nyquist=12000, *10)
    z     = w2 @ leaky_relu(w1 * u + b1, 0.2) + b2   (pointwise MLP, hidden=64)
    alpha = 0.98 * sigmoid(z) * voiced_mask
    y[t]  = alpha[t] * y[t-1] + (1 - alpha[t]) * x[t]   (first-order IIR)

v2 strategy (8 cores, batch-parallel, 2 batch elements per core):
  * alpha(f0) is approximated DIRECTLY (no sigmoid, no MLP on device) by an
    even polynomial in s = f0/fmax: alpha ~= c0 + c1*q + c2*q^2, q = s^2,
    fit on the host (Chebyshev in q).  The voiced mask rides in the sign of
    the shipped value fs = +-s.  One custom DVE instruction (ALPHA_Q2_MASK)
    evaluates alpha = poly(q) * (fs > 0) in a single pass.
  * Per batch element, only 3 full-length DVE passes remain:
        A  = ALPHA_Q2_MASK(fs)                [custom DVE]
        NB = (A - 1) * x                      [scalar_tensor_tensor]
        y  = scan: state = A*state - NB       [tensor_tensor_scan, fp32 state]
  * All HBM I/O is 16-bit: fs bf16, x bf16, y bf16 (host up-casts y).
    DMA per core: 2 x (1 + 1 + 1) MiB = 6 MiB vs 10.5 MiB for the fp32 mix.
  * T = 524288 is laid out [128 partitions x 4096]; each partition scans its
    own chunk with a per-partition initial value (row 0 = initial_state).
    alpha <= amax << 1, so the alpha-product over a 4096 chunk underflows to
    exactly 0 in fp32 and the true carry into partition p is the last scan
    value of partition p-1.  A cumprod over the first `win` columns applies
    the carry:  y[p, i] += cumprod_alpha[p, i] * carry[p],  i < win.
"""

import numpy as np

_B, _T = 16, 524288
_NCORES = 8
_BPC = _B // _NCORES
_P = 128
_L = _T // _P
_NYQ = 12000.0
_K = 10.0 / _NYQ


def _exact_alpha(f0, w1, b1, w2, b2):
    """Reference alpha(f0) in float64 on the host (f0: 1-D array)."""
    u = np.log2(f0 * _K + 1.0)
    h = w1.reshape(-1, 1).astype(np.float64) * u[None, :] + b1.reshape(-1, 1).astype(
        np.float64
    )
    h = np.where(h >= 0.0, h, 0.2 * h)
    z = w2.reshape(-1).astype(np.float64) @ h + float(np.asarray(b2).reshape(-1)[0])
    return 0.98 / (1.0 + np.exp(-z))


def _fit_alpha_q2(w1, b1, w2, b2, fmin, fmax):
    """Fit alpha(f0) ~= c0 + c1*q + c2*q^2 with q = (f0/fmax)^2.

    Returns (c [3], amax, fit_err) where fit_err includes fp32 Horner
    rounding on a dense grid.
    """
    grid = np.linspace(fmin, fmax, 200001)
    ag = _exact_alpha(grid, w1, b1, w2, b2)
    q = (grid / fmax) ** 2
    ch = np.polynomial.chebyshev.Chebyshev.fit(q, ag, 2)
    p = np.polynomial.chebyshev.cheb2poly(ch.convert().coef)
    c = [float(p[k]) if k < len(p) else 0.0 for k in range(3)]
    qf = ((grid / fmax).astype(np.float32) ** 2).astype(np.float32)
    acc = (np.float32(c[2]) * qf + np.float32(c[1])).astype(np.float32)
    acc = (acc * qf + np.float32(c[0])).astype(np.float32)
    err = float(np.abs(acc.astype(np.float64) - ag).max())
    return c, float(ag.max()), err


def _register_alpha_op():
    """Register the ALPHA_Q2_MASK custom DVE op (idempotent).

    out = (imm2*q + s1)*q + s0) * (in0 > 0),  q = in0^2.
    """
    import concourse.dve_ops as dve_ops
    from concourse.dve_spec import Spec, Src0, C0, C1, C2, Zero, sq
    from concourse.dve_spec import lower as dve_lower
    from concourse.dve_uop import DveOpSpec

    NAME = "ALPHA_Q2_MASK"
    for op in dve_ops.OPS:
        if op.name == NAME:
            return op
    q = sq(Src0)
    body = ((C2 * q + C1) * q + C0) * (Src0 > Zero)

    def ref(in0, in1, s0, s1, imm2):
        x = np.asarray(in0, np.float32)
        qq = x * x
        return ((imm2 * qq + s1) * qq + s0) * (x > 0)

    spec = Spec(body=body, reference=ref)
    row = dve_ops._CUSTOM_DVE_ROW_BASE + len(dve_ops.OPS)
    assert row < 0x20
    dve_ops._SUB_OPCODE_FOR_NAME[NAME] = row
    shas = {}
    for ver in ("v3", "v4"):
        s = DveOpSpec(name=NAME, opcode=row, uops=dve_lower(spec, ver=ver), rd1_en=False)
        shas[ver] = s.sha(ver)
    op = dve_ops.DveOp(NAME, spec, False, shas)
    dve_ops.OPS.append(op)
    dve_ops.CUSTOM_DVE_SPECS[NAME] = spec
    return op


def _build_bass_v2(c, win, nchunk=2, reps=1, nb_pool_chunks=(), fs_fp8=False, split_loads=1, nb_tt=False, store_split=True):
    """Per-core Bass program (same program for all cores).

    c: [c0, c1, c2] of the alpha poly in q.  All big tiles bf16.
    """
    import contextlib

    import concourse.mybir as mybir
    from concourse.bacc import Bacc
    from concourse.tile import TileContext

    op = _register_alpha_op()

    f32 = mybir.dt.float32
    bf16 = mybir.dt.bfloat16
    Alu = mybir.AluOpType
    P, L, bpc = _P, _L, _BPC
    CW = L // nchunk

    nc = Bacc()
    fs_dt = mybir.dt.float8e4 if fs_fp8 else bf16
    fs_d = nc.declare_dram_parameter("fs", [bpc, P, L], fs_dt, False)
    x_d = nc.declare_dram_parameter("xin", [bpc, P, L], bf16, False)
    y0_d = nc.declare_dram_parameter("y0", [bpc, P, 1], f32, False)
    y_d = nc.declare_dram_parameter("y", [bpc, P, L], bf16, True)

    with TileContext(nc) as tc:
        with (
            tc.tile_pool(name="big", bufs=2) as pool,
            tc.tile_pool(name="small", bufs=2) as spool,
        ):
            zeros_w = spool.tile([P, win], f32, tag="zw", bufs=1)
            nc.vector.memset(zeros_w, 0.0)

            rep_ctx = tc.For_i(0, reps, 1) if reps > 1 else contextlib.nullcontext()
            with rep_ctx:
                for e in range(bpc):
                    INIT = spool.tile([P, 1], f32, tag="init", bufs=bpc)
                    nc.sync.dma_start(out=INIT, in_=y0_d[e])
                    INITV = spool.tile([P, 1], f32, tag="initv", bufs=bpc)
                    nc.vector.tensor_scalar_mul(INITV, INIT, 1.0)  # absorb DMA wait

                    TFS = pool.tile([P, L], fs_dt, tag="tfs", bufs=2)
                    TX = pool.tile([P, L], bf16, tag="tx", bufs=2)
                    if split_loads == 1:
                        nc.sync.dma_start(out=TFS, in_=fs_d[e])
                        nc.sync.dma_start(out=TX, in_=x_d[e])
                    else:
                        SL = L // split_loads
                        for si in range(split_loads):
                            nc.sync.dma_start(
                                out=TFS[:, si * SL : (si + 1) * SL],
                                in_=fs_d[e][:, si * SL : (si + 1) * SL],
                            )
                        for si in range(split_loads):
                            nc.sync.dma_start(
                                out=TX[:, si * SL : (si + 1) * SL],
                                in_=x_d[e][:, si * SL : (si + 1) * SL],
                            )
                    ZW = pool.tile([P, L], bf16, tag="zwf", bufs=2)

                    a64 = None
                    z_prev = None
                    for ci in range(nchunk):
                        lo, hi = ci * CW, (ci + 1) * CW
                        A = pool.tile([P, CW], bf16, tag="a", bufs=2 * nchunk)
                        nc.vector._custom_dve(
                            op, out=A, in0=TFS[:, lo:hi],
                            s0=c[0], s1=c[1], imm2=c[2],
                        )
                        NB = TX[:, lo:hi]
                        if ci in nb_pool_chunks:
                            T1 = pool.tile([P, CW], bf16, tag="t1p", bufs=2 * nchunk)
                            nc.gpsimd.tensor_tensor(out=T1, in0=A, in1=NB, op=Alu.mult)
                            nc.gpsimd.tensor_tensor(out=NB, in0=T1, in1=NB, op=Alu.subtract)
                        elif nb_tt:
                            # two bf16 tensor_tensor ops run in the DVE 2x
                            # perf mode (0.39 ns/col each vs 1.04 for the stt)
                            T1 = pool.tile([P, CW], bf16, tag="t1", bufs=2 * nchunk)
                            nc.vector.tensor_tensor(out=T1, in0=A, in1=NB, op=Alu.mult)
                            nc.vector.tensor_tensor(out=T1, in0=T1, in1=NB, op=Alu.subtract)
                            NB = T1
                        else:
                            nc.vector.scalar_tensor_tensor(
                                out=NB, in0=A, scalar=1.0, in1=NB,
                                op0=Alu.subtract, op1=Alu.mult,
                            )
                        init = INITV if ci == 0 else z_prev[:, CW - 1 : CW]
                        nc.vector.tensor_tensor_scan(
                            out=ZW[:, lo:hi], data0=A, data1=NB, initial=init,
                            op0=Alu.mult, op1=Alu.subtract,
                        )
                        if ci == 0:
                            a64 = spool.tile([P, win], f32, tag="a64", bufs=bpc)
                            nc.vector.tensor_tensor_scan(
                                out=a64, data0=A[:, :win], data1=zeros_w,
                                initial=1.0, op0=Alu.mult, op1=Alu.add,
                            )
                        z_prev = ZW[:, lo:hi]

                    if store_split:
                        # body store (everything past the correction window)
                        nc.scalar.dma_start(out=y_d[e][:, win:], in_=ZW[:, win:])
                    # carry[p] = last scan value of partition p-1 (p>=1)
                    C = spool.tile([P, 1], bf16, tag="carry", bufs=bpc)
                    nc.vector.memset(C, 0.0)
                    nc.sync.dma_start(out=C[1:P, :], in_=ZW[0 : P - 1, L - 1 : L])
                    nc.vector.scalar_tensor_tensor(
                        out=ZW[:, :win], in0=a64, scalar=C, in1=ZW[:, :win],
                        op0=Alu.mult, op1=Alu.add,
                    )
                    if store_split:
                        nc.scalar.dma_start(out=y_d[e][:, :win], in_=ZW[:, :win])
                    else:
                        nc.scalar.dma_start(out=y_d[e], in_=ZW)
    return nc


def _numpy_fallback(x, f0, vm, y0, w1, b1, w2, b2):
    """Exact (sequential, fp32) host computation.  Safety net only."""
    f32 = np.float32
    alpha = (
        _exact_alpha(f0.reshape(-1).astype(np.float64), w1, b1, w2, b2).reshape(f0.shape)
        * vm.astype(np.float64)
    ).astype(f32)
    beta = ((f32(1.0) - alpha) * x.astype(f32)).astype(f32)
    B = x.shape[0]
    T = x.shape[-1]
    st = y0.reshape(B).astype(f32).copy()
    y = np.empty_like(x, dtype=f32)
    a2 = alpha.reshape(B, T)
    b2_ = beta.reshape(B, T)
    yv = y.reshape(B, T)
    for t in range(T):
        st = (a2[:, t] * st + b2_[:, t]).astype(f32)
        yv[:, t] = st
    return y.reshape(x.shape)


def _pack_inputs(x, f0, vm, y0, fmax):
    """Host-side packing: per-core in_maps with bf16 fs/x and padded y0."""
    import ml_dtypes

    in_maps = []
    for cidx in range(_NCORES):
        sl = slice(cidx * _BPC, (cidx + 1) * _BPC)
        y0pad = np.zeros((_BPC, _P, 1), dtype=np.float32)
        y0pad[:, 0, 0] = y0[sl, 0, 0]
        f0s = f0[sl, 0].reshape(_BPC, _P, _L)
        vms = vm[sl, 0].reshape(_BPC, _P, _L)
        s = f0s * (1.0 / fmax)
        fs = np.where(vms > 0.5, s, -s).astype(ml_dtypes.bfloat16)
        xs = x[sl, 0].reshape(_BPC, _P, _L).astype(ml_dtypes.bfloat16)
        in_maps.append({"fs": fs, "xin": xs, "y0": y0pad})
    return in_maps


def kernel(x, f0_upsampled, voiced_mask, initial_state, w1, b1, w2, b2):
    x = np.ascontiguousarray(np.asarray(x, dtype=np.float32))
    f0 = np.ascontiguousarray(np.asarray(f0_upsampled, dtype=np.float32))
    vm = np.ascontiguousarray(np.asarray(voiced_mask, dtype=np.float32))
    y0 = np.ascontiguousarray(np.asarray(initial_state, dtype=np.float32))
    w1 = np.asarray(w1, dtype=np.float32)
    b1 = np.asarray(b1, dtype=np.float32)
    w2 = np.asarray(w2, dtype=np.float32)
    b2 = np.asarray(b2, dtype=np.float32)

    assert x.shape == (_B, 1, _T), x.shape

    fmin, fmax = float(f0.min()), float(f0.max())
    c, amax, aerr = _fit_alpha_q2(w1, b1, w2, b2, fmin, fmax)

    # The bf16/deg-2 fast path needs a small, well-fit alpha.  (For the
    # reference weights: amax ~ 0.017, aerr ~ 2.6e-4.)
    if not (fmin > 0.0 and amax <= 0.35 and aerr <= 1.5e-3):
        return _numpy_fallback(x, f0, vm, y0, w1, b1, w2, b2)

    win = int(np.ceil(150.0 * np.log(2.0) / -np.log(max(amax, 1e-6))))
    win = int(min(max(64, ((win + 31) // 32) * 32), 512))

    nc = _build_bass_v2(c, win)
    nc.finalize()

    from concourse.bass_utils import run_bass_kernel_spmd

    in_maps = _pack_inputs(x, f0, vm, y0, fmax)
    res = run_bass_kernel_spmd(nc, in_maps, list(range(_NCORES)))
    out = np.empty((_B, 1, _T), dtype=np.float32)
    for cidx in range(_NCORES):
        out[cidx * _BPC : (cidx + 1) * _BPC, 0] = (
            res.results[cidx]["y"].astype(np.float32).reshape(_BPC, _T)
        )
    return out
